# revision 7
# baseline (speedup 1.0000x reference)
"""Trainium2 Bass kernel for nn_AverageAttn_62981400428866.

Reference computation (B=4, S=2048, D=1024):
    avg = cumavg_s(iV)                      # AAN lower-tri 1/(i+1) attention
    h   = relu(avg @ W1 + b1)
    ffn = h @ W2 + b2
    g   = sigmoid(concat([iQ, ffn], -1) @ Wg + bg)
    out = g[..., :D] * iQ + g[..., D:] * ffn

Sharding: 8 cores <- (batch b = c//2, seq half h = c%2); each core owns 1024
tokens. The cumulative-sum carry for h=1 cores (sum of the first-half iV) is
precomputed on the host and shipped in the f32 const blob (h=0 gets zeros).

HW cost model (probe-calibrated): every matmul costs ~N_moving x 0.4167 ns
on the PE regardless of dtype/perf_mode (DoubleRow buys K=256 packing, not
row rate), so total PE time ~= total moving rows. The design minimizes rows:
  - attn runs entirely on the DVE via tensor_tensor_scan (f16 V, f32 state,
    exact cumsum + carry chaining) - zero PE rows, and more accurate than
    any fp8 matmul cumsum.
  - mm1/mm2: token cols 64+ in fp8e4m3 DoubleRow (pairs of K-tiles per
    instruction, scales folded into pre-quantized weights); cols 0:64 in
    fp16 (large-|ffn| early tokens dominate absmax error; 64 cols measured
    equivalent to 128 on this dataset).
  - gate: iQ half as xh*(wgh + wgl) fp8 DR (weight hi/lo residual pair,
    full K - any stream trim pushes absmax past the 2e-2 gate); ffn half
    plain fp8 DR (ffn8 = ffn*32, wg8 = Wg_bot*128). Every term lands at
    psum scale 4096; one Sigmoid eviction rescales by 1/4096.
  - final: f16 gates/operands and f16 output (2x DVE rate), written
    feature-major [D, tok] per core, cast + transposed on the host.
Config measures 8.29e-3 L2 / 1.43e-2 absmax on the fixed dataset (gate
rel_err < 2e-2).

A post-compile pass drops InstLdweights whose weights AP matches the
previous load on the in-order PE queue (~24 ns each on HW).

DMA discipline: transfers are batched (const blobs, whole-tensor activation
loads, mc-paired mm weights, dc-paired gate weights) and ordered by first
consumption. The reps parameter repeats the body for (T_n - T_1)/(n-1) HW
timing only.
"""

import numpy as np
import ml_dtypes

B, S, D = 4, 2048, 1024
P = 128
NCORES = 8
TOK = S // 2          # tokens per core
TT = TOK // P         # token tiles per core
KC = D // P           # feature chunks
GC = 2 * D // P       # gate-dim chunks
NT = 512              # matmul moving free dim
NN = TOK // NT
F16C = 32             # fp16-protected token columns in mm1/mm2

S_A = 32.0            # avg8 scale
S_W1 = 1024.0         # w1_8 scale
S_H = 16.0            # h8 scale
S_W2 = 1024.0         # w2_8 scale
S_F = 32.0            # ffn8 scale
S_WG = 128.0          # wg8 (bottom) scale
S_IQ = 16.0           # iq scale
S_WGT = 256.0         # wg top hi/lo scale
SCALE_G = S_F * S_WG  # gate psum scale (= S_IQ * S_WGT)

E4 = ml_dtypes.float8_e4m3
_CACHE = {}

# const blob column offsets (f16 blob: inv | inv8 ; f32 blob:
# carry | b1(2,KC) | b2(2,KC) | bg)
C16_W = 2 * TOK
C32_W = KC + 2 * KC + 2 * KC + GC


def _dedup_ldweights(nc):
    """Drop InstLdweights identical to the previous PE weight load.

    The PE array keeps its stationary weights between matmuls; a reload
    with a byte-identical AP is redundant. Invalidate the cached
    signature on any other PE instruction, on any instruction writing
    the weight memref (tile-buffer reuse), and never drop a load that
    carries sync info.
    """
    import concourse.mybir as mybir

    PE = mybir.EngineType.PE
    dropped = 0
    for fn in nc.m.functions:
        for blk in fn.blocks:
            out = []
            last_sig = None
            last_memref = None
            for inst in blk.instructions:
                nm = type(inst).__name__
                if nm == "InstLdweights":
                    ap = inst.ins[0]
                    sig = (
                        ap.memref,
                        ap.offset,
                        str(ap.ap),
                        str(ap.dtype),
                        str(inst.perf_mode),
                        inst.is_transpose,
                        inst.tile_position,
                        inst.tile_size,
                    )
                    si = inst.sync_info
                    clean = si is None or (
                        not si.on_wait and not si.on_update
                    )
                    if clean and sig == last_sig:
                        dropped += 1
                        continue
                    last_sig = sig
                    last_memref = ap.memref
                    out.append(inst)
                    continue
                if nm == "InstMatmult":
                    out.append(inst)
                    continue
                if getattr(inst, "engine", None) == PE:
                    last_sig = None
                elif last_memref is not None:
                    for o in inst.outs:
                        if getattr(o, "memref", None) == last_memref:
                            last_sig = None
                            break
                out.append(inst)
            blk.instructions[:] = out
    return dropped


def _build_nc(reps=1, stages=("attn", "mm1", "mm2", "gate")):
    from concourse import bacc
    import concourse.mybir as mybir
    from concourse.tile import TileContext

    f32 = mybir.dt.float32
    f16 = mybir.dt.float16
    f8 = mybir.dt.float8e4
    AF = mybir.ActivationFunctionType
    ALU = mybir.AluOpType
    DR = mybir.MatmulPerfMode.DoubleRow

    nc = bacc.Bacc(None, target_bir_lowering=False)
    iq_d = nc.dram_tensor("iq", [D, TOK], f16, kind="ExternalInput")
    iqh_d = nc.dram_tensor("iqh", [D, TOK], f8, kind="ExternalInput")
    ivt_d = nc.dram_tensor("ivt", [P, KC, TOK], f16, kind="ExternalInput")
    w116_d = nc.dram_tensor("w116", [KC // 2, P, 2, KC, P], f16,
                            kind="ExternalInput")
    w18_d = nc.dram_tensor("w18", [KC // 2, P, 2, KC // 2, 2, P], f8,
                           kind="ExternalInput")
    w216_d = nc.dram_tensor("w216", [KC // 2, P, 2, KC, P], f16,
                            kind="ExternalInput")
    w28_d = nc.dram_tensor("w28", [KC // 2, P, 2, KC // 2, 2, P], f8,
                           kind="ExternalInput")
    wgh_d = nc.dram_tensor("wgh", [KC, P, 2, KC // 2, 2, P], f8,
                           kind="ExternalInput")
    wgl_d = nc.dram_tensor("wgl", [KC, P, 2, KC // 2, 2, P], f8,
                           kind="ExternalInput")
    wgb_d = nc.dram_tensor("wgb", [KC, P, 2, KC // 2, 2, P], f8,
                           kind="ExternalInput")
    c16_d = nc.dram_tensor("c16", [P, C16_W], f16, kind="ExternalInput")
    c32_d = nc.dram_tensor("c32", [P, C32_W], f32, kind="ExternalInput")
    out_d = nc.dram_tensor("outT", [D, TOK], f16, kind="ExternalOutput")
    scratch_d = (
        nc.dram_tensor("scratch", [D, TOK], f16, kind="Internal")
        if reps > 1
        else None
    )

    with TileContext(nc) as tc:
        with (
            tc.tile_pool(name="big", bufs=1) as big,
            tc.tile_pool(name="wpool", bufs=4) as wpool,
            tc.tile_pool(name="tok", bufs=4) as tokp,
            tc.tile_pool(name="gp", bufs=3) as gpool,
            tc.tile_pool(name="const", bufs=1) as constp,
            tc.tile_pool(name="mm_ps", bufs=8, space="PSUM") as mm_ps,
        ):
            # consts are rep-invariant: load once
            c16 = constp.tile([P, C16_W], f16)
            inv_t = c16[:, 0:TOK]
            inv8_t = c16[:, TOK : 2 * TOK]
            c32 = constp.tile([P, C32_W], f32)
            nc.sync.dma_start(c32[:], c32_d[:])
            nc.sync.dma_start(c16[:], c16_d[:])
            carry = c32[:, 0:KC]
            # bias views into the f32 blob
            b1_col = lambda i, mc: c32[:, KC + i * KC + mc : KC + i * KC + mc + 1]
            b2_col = lambda i, mc: c32[:, 3 * KC + i * KC + mc : 3 * KC + i * KC + mc + 1]
            bg_col = lambda gc: c32[:, 5 * KC + gc : 5 * KC + gc + 1]

            def emit_attn():
                """V load + DVE cumsum scans + avg muls for one rep.

                Called at the top for rep 0 and MID-GATE of rep r for
                rep r+1 (software pipelining): the scans then run on the
                DVE while the PE is busy with gate matmuls, so the next
                rep's mm1 is never scan-gated.
                """
                V = big.tile([P, KC, TOK], f16, tag="V", bufs=2)
                nc.sync.dma_start(V[:, 0:3], ivt_d[:, 0:3])
                nc.sync.dma_start(V[:, 3:], ivt_d[:, 3:])
                cums = big.tile([P, KC, TOK], f16, tag="cums")
                avgT = big.tile([P, KC, F16C], f16, tag="avgT")
                avg8 = big.tile([P, KC, TOK], f8, tag="avg8")
                for dc in range(KC):
                    nc.vector.tensor_tensor_scan(
                        cums[:, dc], V[:, dc], V[:, dc],
                        carry[:, dc : dc + 1], ALU.add, ALU.bypass,
                    )
                    nc.vector.tensor_tensor(
                        avg8[:, dc, F16C:], cums[:, dc, F16C:],
                        inv8_t[:, F16C:], ALU.mult
                    )
                    nc.vector.tensor_tensor(
                        avgT[:, dc], cums[:, dc, 0:F16C], inv_t[:, 0:F16C],
                        ALU.mult
                    )
                return (avgT, avg8)

            pending = emit_attn()
            for rep in range(reps):
              avgT, avg8 = pending[0], pending[1]
              pre_w = pending[2:] if len(pending) > 2 else None
              out_rep = out_d if rep == 0 else scratch_d
              if "mm1" not in stages:
                continue
              # ---- mm1: hT/h8 = relu(W1^T @ avg + b1) -------------------
              # fp16 cols 0:F16C (W1 fp16 pre-scaled by S_A*S_W1 so the
              # psum scale is uniform), fp8 DR cols F16C:
              hT = big.tile([P, KC, F16C], f16, tag="hT")
              h8 = big.tile([P, KC, TOK], f8, tag="h8")
              inv_s1 = 1.0 / (S_A * S_W1)
              for q in range(KC // 2):
                if q == 0 and pre_w is not None:
                    w16, w8 = pre_w
                else:
                    w16 = wpool.tile([P, 2, KC, P], f16, tag="w16")
                    nc.sync.dma_start(w16[:], w116_d[q])
                    w8 = wpool.tile([P, 2, KC // 2, 2, P], f8, tag="w8")
                    nc.sync.dma_start(w8[:], w18_d[q])
                for m in range(2):
                    mc = 2 * q + m
                    pss = [
                        mm_ps.tile([P, NT], f32, tag="mps", name=f"mps{i}")
                        for i in range(NN)
                    ]
                    for kc in range(KC):
                        nc.tensor.matmul(
                            pss[0][:, 0:F16C],
                            w16[:, m, kc],
                            avgT[:, kc],
                            start=(kc == 0),
                            stop=(kc == KC - 1),
                        )
                    for j in range(KC // 2):
                        nc.tensor.matmul(
                            pss[0][:, F16C:NT],
                            w8[:, m, j],
                            avg8[:, 2 * j : 2 * j + 2, F16C:NT],
                            start=False,
                            stop=(j == KC // 2 - 1),
                            perf_mode=DR,
                            skip_group_check=True,
                        )
                        nc.tensor.matmul(
                            pss[1][:],
                            w8[:, m, j],
                            avg8[:, 2 * j : 2 * j + 2, NT : 2 * NT],
                            start=(j == 0),
                            stop=(j == KC // 2 - 1),
                            perf_mode=DR,
                            skip_group_check=True,
                        )
                    nc.scalar.activation(
                        hT[:, mc], pss[0][:, 0:F16C], AF.Relu,
                        scale=inv_s1, bias=b1_col(0, mc),
                    )
                    nc.scalar.activation(
                        h8[:, mc, F16C:NT], pss[0][:, F16C:NT], AF.Relu,
                        scale=inv_s1 * S_H, bias=b1_col(1, mc),
                    )
                    nc.scalar.activation(
                        h8[:, mc, NT : 2 * NT], pss[1][:], AF.Relu,
                        scale=inv_s1 * S_H, bias=b1_col(1, mc),
                    )

              if "mm2" not in stages:
                continue
              # ---- mm2: ffnT/ffn8 = W2^T @ h + b2 ----------------------
              ffnT = big.tile([P, KC, TOK], f16, tag="ffnT")
              ffn8 = big.tile([P, KC, TOK], f8, tag="ffn8")
              inv_s2 = 1.0 / (S_H * S_W2)
              for q in range(KC // 2):
                w16 = wpool.tile([P, 2, KC, P], f16, tag="w16")
                nc.sync.dma_start(w16[:], w216_d[q])
                w8 = wpool.tile([P, 2, KC // 2, 2, P], f8, tag="w8")
                nc.sync.dma_start(w8[:], w28_d[q])
                for m in range(2):
                    mc = 2 * q + m
                    pss = [
                        mm_ps.tile([P, NT], f32, tag="mps", name=f"mps{i}")
                        for i in range(NN)
                    ]
                    for kc in range(KC):
                        nc.tensor.matmul(
                            pss[0][:, 0:F16C],
                            w16[:, m, kc],
                            hT[:, kc],
                            start=(kc == 0),
                            stop=(kc == KC - 1),
                        )
                    for j in range(KC // 2):
                        nc.tensor.matmul(
                            pss[0][:, F16C:NT],
                            w8[:, m, j],
                            h8[:, 2 * j : 2 * j + 2, F16C:NT],
                            start=False,
                            stop=(j == KC // 2 - 1),
                            perf_mode=DR,
                            skip_group_check=True,
                        )
                        nc.tensor.matmul(
                            pss[1][:],
                            w8[:, m, j],
                            h8[:, 2 * j : 2 * j + 2, NT : 2 * NT],
                            start=(j == 0),
                            stop=(j == KC // 2 - 1),
                            perf_mode=DR,
                            skip_group_check=True,
                        )
                    for n in range(NN):
                        nsl = slice(n * NT, (n + 1) * NT)
                        nc.scalar.activation(
                            ffnT[:, mc, nsl], pss[n][:], AF.Identity,
                            scale=inv_s2, bias=b2_col(0, mc),
                        )
                    # ffn8 on DVE keeps Act off the critical path
                    nc.vector.tensor_scalar(
                        ffn8[:, mc], ffnT[:, mc], float(S_F), None, ALU.mult
                    )

              # gate-phase inputs: first dc's gate weights ahead of the big
              # activation loads so the gate start isn't DMA-gated
              iQT = big.tile([P, KC, TOK], f16, tag="iQT")
              iqh8 = big.tile([P, KC, TOK], f8, tag="iqh8")
              wg_pre = []
              for nm, dram in (("h", wgh_d), ("l", wgl_d), ("b", wgb_d)):
                w = wpool.tile([P, 2, KC // 2, 2, P], f8, tag=f"wgpre{nm}",
                               bufs=1, name=f"wgp{nm}")
                nc.sync.dma_start(w[:], dram[0])
                wg_pre.append(w)
              iqh_r = iqh_d.rearrange("(dc p) n -> p dc n", p=P)
              nc.sync.dma_start(iqh8[:], iqh_r[:])
              iq_r = iq_d.rearrange("(dc p) n -> p dc n", p=P)
              nc.sync.dma_start(iQT[:], iq_r[:])

              if "gate" not in stages:
                continue
              # ---- gate + final: per dc, gc pair (dc, dc+KC) -----------
              # one psum group per (gc, n): 8 hi + 8 lo iQ DR + 8 ffn DR,
              # all at psum scale 4096; Sigmoid evicts with scale 1/4096.
              for dc in range(KC):
                if dc == 0:
                    wgh, wgl, wgb = wg_pre
                else:
                    wgh = wpool.tile([P, 2, KC // 2, 2, P], f8, tag="wgh")
                    nc.sync.dma_start(wgh[:], wgh_d[dc])
                    wgl = wpool.tile([P, 2, KC // 2, 2, P], f8, tag="wgl")
                    nc.sync.dma_start(wgl[:], wgl_d[dc])
                    wgb = wpool.tile([P, 2, KC // 2, 2, P], f8, tag="wgb")
                    nc.sync.dma_start(wgb[:], wgb_d[dc])
                # software pipeline: mid-gate, queue the NEXT rep's attn
                # on the DVE so its scans overlap this rep's gate matmuls,
                # and issue its first mm1 weight DMAs so the SP queue has
                # them in flight before this rep's tail out-stores
                if dc == 2 and rep + 1 < reps:
                    pending = emit_attn()
                    w16p = wpool.tile([P, 2, KC, P], f16, tag="w16")
                    nc.sync.dma_start(w16p[:], w116_d[0])
                    w8p = wpool.tile([P, 2, KC // 2, 2, P], f8, tag="w8")
                    nc.sync.dma_start(w8p[:], w18_d[0])
                    pending = pending + (w16p, w8p)
                g2 = gpool.tile([P, 2, TOK], f16, tag="g")
                outc = tokp.tile([P, TOK], f16, tag="tok")
                tmp = tokp.tile([P, TOK], f16, tag="tok")
                for gi in range(2):
                    gc = dc + gi * KC
                    pss = [
                        mm_ps.tile([P, NT], f32, tag="mps", name=f"mps{i}")
                        for i in range(NN)
                    ]
                    # per weight chunk j: one Ld serves the n0/n1 pair
                    # (post-compile dedup drops the second reload)
                    for ti, wq in enumerate((wgh, wgl)):
                        for j in range(KC // 2):
                            for n in range(NN):
                                nsl = slice(n * NT, (n + 1) * NT)
                                nc.tensor.matmul(
                                    pss[n][:],
                                    wq[:, gi, j],
                                    iqh8[:, 2 * j : 2 * j + 2, nsl],
                                    start=(ti == 0 and j == 0),
                                    stop=False,
                                    perf_mode=DR,
                                    skip_group_check=True,
                                )
                    for j in range(KC // 2):
                        for n in range(NN):
                            nsl = slice(n * NT, (n + 1) * NT)
                            nc.tensor.matmul(
                                pss[n][:],
                                wgb[:, gi, j],
                                ffn8[:, 2 * j : 2 * j + 2, nsl],
                                start=False,
                                stop=(j == KC // 2 - 1),
                                perf_mode=DR,
                                skip_group_check=True,
                            )
                    # last chunk's fgate sigmoid at 256-col granularity so
                    # the tail's DVE chain starts as soon as possible
                    sub = 2 if (dc == KC - 1 and gi == 1) else 1
                    for n in range(NN):
                        for k in range(sub):
                            w = NT // sub
                            nsl = slice(n * NT + k * w, n * NT + (k + 1) * w)
                            nc.scalar.activation(
                                g2[:, gi, nsl], pss[n][:, k * w : (k + 1) * w],
                                AF.Sigmoid,
                                scale=1.0 / SCALE_G, bias=bg_col(gc),
                            )
                        # igate*iQ overlaps the fgate matmuls on DVE
                        if gi == 0:
                            nsl = slice(n * NT, (n + 1) * NT)
                            nc.vector.tensor_tensor(
                                outc[:, nsl], g2[:, 0, nsl],
                                iQT[:, dc, nsl], ALU.mult
                            )
                # last chunk runs at half-NT granularity to shorten the
                # sigmoid -> DVE -> DMA tail; earlier chunks batch into one
                # store
                steps = 2 * NN if dc == KC - 1 else NN
                W = TOK // steps
                for n in range(steps):
                    nsl = slice(n * W, (n + 1) * W)
                    nc.vector.tensor_tensor(
                        tmp[:, nsl], g2[:, 1, nsl], ffnT[:, dc, nsl],
                        ALU.mult
                    )
                    nc.vector.tensor_tensor(
                        outc[:, nsl], outc[:, nsl], tmp[:, nsl], ALU.add
                    )
                    if dc == KC - 1:
                        nc.sync.dma_start(
                            out_rep[dc * P : (dc + 1) * P, nsl],
                            outc[:, nsl],
                        )
                if dc < KC - 1:
                    nc.sync.dma_start(
                        out_rep[dc * P : (dc + 1) * P, :], outc[:]
                    )

    nc.compile()
    _dedup_ldweights(nc)
    return nc


def _get_nc(reps=1, stages=("attn", "mm1", "mm2", "gate")):
    key = ("nc", reps, tuple(stages))
    if key not in _CACHE:
        _CACHE[key] = _build_nc(reps, stages)
    return _CACHE[key]


def _pack_pairs(W, s):
    """[D_in, P*mc] weight chunk -> fp8 [mc, P(k), pairs, 2, P(m)] * s."""
    din, dout = W.shape
    kc, mc = din // P, dout // P
    r = (W * s).reshape(kc // 2, 2, P, mc, P)  # (pair, i, k, mc, m)
    r = np.ascontiguousarray(r.transpose(3, 2, 0, 1, 4))  # mc, k, pair, i, m
    return r.astype(E4)


def _pack_pairs_pre(Wq):
    """Pre-quantized [D_in, D_out] -> fp8 [mc, P(k), pairs, 2, P(m)]."""
    din, dout = Wq.shape
    kc, mc = din // P, dout // P
    r = Wq.reshape(kc // 2, 2, P, mc, P)
    r = np.ascontiguousarray(r.transpose(3, 2, 0, 1, 4))
    return r.astype(E4)


def _rearr16(W, s=1.0):
    """[D_in, P*mc] -> fp16 [mc, P(k), kc, P(m)] (lhsT chunk layout)."""
    din, dout = W.shape
    kc, mc = din // P, dout // P
    r = (W * s).reshape(kc, P, mc, P).transpose(2, 1, 0, 3)
    return np.ascontiguousarray(r).astype(np.float16)


def _mc_pair(a):
    """[mc, P, ...] -> [mc//2, P, 2, ...] (adjacent output-chunk pairs)."""
    m = a.shape[0]
    r = a.reshape(m // 2, 2, *a.shape[1:])
    return np.ascontiguousarray(np.swapaxes(r, 1, 2))


def _gc_pair(a):
    """[gc(16), P, ...] -> [dc(8), P, 2, ...] pairing (dc, dc+KC)."""
    r = a.reshape(2, KC, *a.shape[1:])            # [gi, dc, P, ...]
    r = np.moveaxis(r, 0, 2)                      # [dc, P, gi, ...]
    return np.ascontiguousarray(r)


def _host_inputs(iQ, iV, W1, b1, W2, b2, Wg, bg):
    iQ = np.asarray(iQ, np.float32)
    iV = np.asarray(iV, np.float32)
    W1 = np.asarray(W1, np.float32)
    W2 = np.asarray(W2, np.float32)
    Wg = np.asarray(Wg, np.float32)

    w116 = _mc_pair(_rearr16(W1, S_A * S_W1))
    w18 = _mc_pair(_pack_pairs(W1, S_W1))
    w216 = _mc_pair(_rearr16(W2, S_H * S_W2))
    w28 = _mc_pair(_pack_pairs(W2, S_W2))
    wgt_s = Wg[:D] * S_WGT
    wgth_v = wgt_s.astype(E4)
    wgh = _gc_pair(_pack_pairs_pre(wgth_v.astype(np.float32)))
    wgl = _gc_pair(_pack_pairs_pre(
        (wgt_s - wgth_v.astype(np.float32)).astype(E4).astype(np.float32)
    ))
    wgb = _gc_pair(_pack_pairs(Wg[D:], S_WG))

    def bias2(b, s):
        bc = np.asarray(b, np.float32).reshape(KC, P).T  # [P, KC]
        return np.stack([bc, bc * s], axis=1)  # [P, 2, KC]

    b1c = bias2(b1, S_H)
    b2c = bias2(b2, S_F)
    bgc = np.asarray(bg, np.float32).reshape(GC, P).T

    in_maps = []
    for c in range(NCORES):
        b, h = divmod(c, 2)
        sl = slice(h * TOK, (h + 1) * TOK)
        inv = np.float32(1.0) / np.arange(
            h * TOK + 1, h * TOK + TOK + 1, dtype=np.float32
        )
        carry0 = (
            iV[b, :TOK].astype(np.float64).sum(axis=0).astype(np.float32)
            if h
            else np.zeros(D, np.float32)
        )
        c16 = np.empty((P, C16_W), np.float16)
        c16[:, 0:TOK] = inv.astype(np.float16)[None, :]
        c16[:, TOK:] = (inv * S_A).astype(np.float16)[None, :]
        c32 = np.empty((P, C32_W), np.float32)
        c32[:, 0:KC] = carry0.reshape(KC, P).T
        c32[:, KC : 3 * KC] = b1c.reshape(P, 2 * KC)
        c32[:, 3 * KC : 5 * KC] = b2c.reshape(P, 2 * KC)
        c32[:, 5 * KC :] = bgc
        # V feature-major f16: ivt[p, kc, t] = iV[b, sl][t, kc*P+p]
        ivt = np.ascontiguousarray(
            iV[b, sl].T.reshape(KC, P, TOK).transpose(1, 0, 2)
        ).astype(np.float16)
        iqs = iQ[b, sl].T * S_IQ  # [D, TOK]
        in_maps.append(
            {
                "iq": np.ascontiguousarray(iQ[b, sl].T).astype(np.float16),
                "iqh": np.ascontiguousarray(iqs.astype(E4)),
                "ivt": ivt,
                "w116": w116,
                "w18": w18,
                "w216": w216,
                "w28": w28,
                "wgh": wgh,
                "wgl": wgl,
                "wgb": wgb,
                "c16": c16,
                "c32": c32,
            }
        )
    return in_maps


def _gather(results):
    out = np.empty((B, S, D), np.float32)
    for c in range(NCORES):
        b, h = divmod(c, 2)
        out[b, h * TOK : (h + 1) * TOK, :] = (
            results[c]["outT"].astype(np.float32).T
        )
    return out


def kernel(iQ, iV, W1, b1, W2, b2, Wg, bg):
    from concourse.bass_utils import run_bass_kernel_spmd

    nc = _get_nc()
    in_maps = _host_inputs(iQ, iV, W1, b1, W2, b2, Wg, bg)
    res = run_bass_kernel_spmd(nc, in_maps, core_ids=list(range(NCORES)))
    return _gather(res.results)


# revision 8
# speedup vs baseline: 1.0071x; 1.0071x over previous
"""Trainium2 Bass kernel for nn_AverageAttn_62981400428866.

Reference computation (B=4, S=2048, D=1024):
    avg = cumavg_s(iV)                      # AAN lower-tri 1/(i+1) attention
    h   = relu(avg @ W1 + b1)
    ffn = h @ W2 + b2
    g   = sigmoid(concat([iQ, ffn], -1) @ Wg + bg)
    out = g[..., :D] * iQ + g[..., D:] * ffn

Sharding: 8 cores <- (batch b = c//2, seq half h = c%2); each core owns 1024
tokens. The cumulative-sum carry for h=1 cores (sum of the first-half iV) is
precomputed on the host and shipped in the f32 const blob (h=0 gets zeros).

HW cost model (probe-calibrated): every matmul costs ~N_moving x 0.4167 ns
on the PE regardless of dtype/perf_mode (DoubleRow buys K=256 packing, not
row rate), so total PE time ~= total moving rows. The design minimizes rows:
  - attn runs entirely on the DVE via tensor_tensor_scan (f16 V, f32 state,
    exact cumsum + carry chaining) - zero PE rows, and more accurate than
    any fp8 matmul cumsum.
  - mm1/mm2: token cols 64+ in fp8e4m3 DoubleRow (pairs of K-tiles per
    instruction, scales folded into pre-quantized weights); cols 0:64 in
    fp16 (large-|ffn| early tokens dominate absmax error; 64 cols measured
    equivalent to 128 on this dataset).
  - gate: iQ half as xh*(wgh + wgl) fp8 DR (weight hi/lo residual pair,
    full K - any stream trim pushes absmax past the 2e-2 gate); ffn half
    plain fp8 DR (ffn8 = ffn*32, wg8 = Wg_bot*128). Every term lands at
    psum scale 4096; one Sigmoid eviction rescales by 1/4096.
  - final: f16 gates/operands and f16 output (2x DVE rate), written
    feature-major [D, tok] per core, cast + transposed on the host.
Config measures 8.29e-3 L2 / 1.43e-2 absmax on the fixed dataset (gate
rel_err < 2e-2).

A post-compile pass drops InstLdweights whose weights AP matches the
previous load on the in-order PE queue (~24 ns each on HW).

DMA discipline: transfers are batched (const blobs, whole-tensor activation
loads, mc-paired mm weights, dc-paired gate weights) and ordered by first
consumption. The reps parameter repeats the body for (T_n - T_1)/(n-1) HW
timing only.
"""

import numpy as np
import ml_dtypes

B, S, D = 4, 2048, 1024
P = 128
NCORES = 8
TOK = S // 2          # tokens per core
TT = TOK // P         # token tiles per core
KC = D // P           # feature chunks
GC = 2 * D // P       # gate-dim chunks
NT = 512              # matmul moving free dim
NN = TOK // NT
F16C = 32             # fp16-protected token columns in mm1/mm2

S_A = 32.0            # avg8 scale
S_W1 = 1024.0         # w1_8 scale
S_H = 16.0            # h8 scale
S_W2 = 1024.0         # w2_8 scale
S_F = 32.0            # ffn8 scale
S_WG = 128.0          # wg8 (bottom) scale
S_IQ = 16.0           # iq scale
S_WGT = 256.0         # wg top hi/lo scale
SCALE_G = S_F * S_WG  # gate psum scale (= S_IQ * S_WGT)

E4 = ml_dtypes.float8_e4m3
_CACHE = {}

# const blob column offsets (f16 blob: inv | inv8 ; f32 blob:
# carry | b1(2,KC) | b2(2,KC) | bg)
C16_W = 2 * TOK
C32_W = KC + 2 * KC + 2 * KC + GC


def _dedup_ldweights(nc):
    """Drop InstLdweights identical to the previous PE weight load.

    The PE array keeps its stationary weights between matmuls; a reload
    with a byte-identical AP is redundant. Invalidate the cached
    signature on any other PE instruction, on any instruction writing
    the weight memref (tile-buffer reuse), and never drop a load that
    carries sync info.
    """
    import concourse.mybir as mybir

    PE = mybir.EngineType.PE
    dropped = 0
    for fn in nc.m.functions:
        for blk in fn.blocks:
            out = []
            last_sig = None
            last_memref = None
            for inst in blk.instructions:
                nm = type(inst).__name__
                if nm == "InstLdweights":
                    ap = inst.ins[0]
                    sig = (
                        ap.memref,
                        ap.offset,
                        str(ap.ap),
                        str(ap.dtype),
                        str(inst.perf_mode),
                        inst.is_transpose,
                        inst.tile_position,
                        inst.tile_size,
                    )
                    si = inst.sync_info
                    clean = si is None or (
                        not si.on_wait and not si.on_update
                    )
                    if clean and sig == last_sig:
                        dropped += 1
                        continue
                    last_sig = sig
                    last_memref = ap.memref
                    out.append(inst)
                    continue
                if nm == "InstMatmult":
                    out.append(inst)
                    continue
                if getattr(inst, "engine", None) == PE:
                    last_sig = None
                elif last_memref is not None:
                    for o in inst.outs:
                        if getattr(o, "memref", None) == last_memref:
                            last_sig = None
                            break
                out.append(inst)
            blk.instructions[:] = out
    return dropped


def _build_nc(reps=1, stages=("attn", "mm1", "mm2", "gate")):
    from concourse import bacc
    import concourse.mybir as mybir
    from concourse.tile import TileContext

    f32 = mybir.dt.float32
    f16 = mybir.dt.float16
    f8 = mybir.dt.float8e4
    AF = mybir.ActivationFunctionType
    ALU = mybir.AluOpType
    DR = mybir.MatmulPerfMode.DoubleRow

    nc = bacc.Bacc(None, target_bir_lowering=False)
    iq_d = nc.dram_tensor("iq", [D, TOK], f16, kind="ExternalInput")
    iqh_d = nc.dram_tensor("iqh", [D, TOK], f8, kind="ExternalInput")
    ivt_d = nc.dram_tensor("ivt", [P, KC, TOK], f16, kind="ExternalInput")
    w116_d = nc.dram_tensor("w116", [KC // 2, P, 2, KC, P], f16,
                            kind="ExternalInput")
    w18_d = nc.dram_tensor("w18", [KC // 2, P, 2, KC // 2, 2, P], f8,
                           kind="ExternalInput")
    w216_d = nc.dram_tensor("w216", [KC // 2, P, 2, KC, P], f16,
                            kind="ExternalInput")
    w28_d = nc.dram_tensor("w28", [KC // 2, P, 2, KC // 2, 2, P], f8,
                           kind="ExternalInput")
    wgh_d = nc.dram_tensor("wgh", [KC, P, 2, KC // 2, 2, P], f8,
                           kind="ExternalInput")
    wgl_d = nc.dram_tensor("wgl", [KC, P, 2, KC // 2, 2, P], f8,
                           kind="ExternalInput")
    wgb_d = nc.dram_tensor("wgb", [KC, P, 2, KC // 2, 2, P], f8,
                           kind="ExternalInput")
    c16_d = nc.dram_tensor("c16", [P, C16_W], f16, kind="ExternalInput")
    c32_d = nc.dram_tensor("c32", [P, C32_W], f32, kind="ExternalInput")
    out_d = nc.dram_tensor("outT", [D, TOK], f16, kind="ExternalOutput")
    scratch_d = (
        nc.dram_tensor("scratch", [D, TOK], f16, kind="Internal")
        if reps > 1
        else None
    )

    with TileContext(nc) as tc:
        with (
            tc.tile_pool(name="big", bufs=1) as big,
            tc.tile_pool(name="wpool", bufs=4) as wpool,
            tc.tile_pool(name="tok", bufs=4) as tokp,
            tc.tile_pool(name="gp", bufs=3) as gpool,
            tc.tile_pool(name="const", bufs=1) as constp,
            tc.tile_pool(name="mm_ps", bufs=8, space="PSUM") as mm_ps,
        ):
            # consts are rep-invariant: load once
            c16 = constp.tile([P, C16_W], f16)
            inv_t = c16[:, 0:TOK]
            inv8_t = c16[:, TOK : 2 * TOK]
            c32 = constp.tile([P, C32_W], f32)
            nc.sync.dma_start(c32[:], c32_d[:])
            nc.sync.dma_start(c16[:], c16_d[:])
            carry = c32[:, 0:KC]
            # bias views into the f32 blob
            b1_col = lambda i, mc: c32[:, KC + i * KC + mc : KC + i * KC + mc + 1]
            b2_col = lambda i, mc: c32[:, 3 * KC + i * KC + mc : 3 * KC + i * KC + mc + 1]
            bg_col = lambda gc: c32[:, 5 * KC + gc : 5 * KC + gc + 1]

            def emit_attn():
                """V load + DVE cumsum scans + avg muls for one rep.

                Called at the top for rep 0 and MID-GATE of rep r for
                rep r+1 (software pipelining): the scans then run on the
                DVE while the PE is busy with gate matmuls, so the next
                rep's mm1 is never scan-gated.
                """
                V = big.tile([P, KC, TOK], f16, tag="V", bufs=2)
                nc.sync.dma_start(V[:, 0:3], ivt_d[:, 0:3])
                nc.sync.dma_start(V[:, 3:], ivt_d[:, 3:])
                cums = big.tile([P, KC, TOK], f16, tag="cums")
                avgT = big.tile([P, KC, F16C], f16, tag="avgT")
                avg8 = big.tile([P, KC, TOK], f8, tag="avg8")
                for dc in range(KC):
                    nc.vector.tensor_tensor_scan(
                        cums[:, dc], V[:, dc], V[:, dc],
                        carry[:, dc : dc + 1], ALU.add, ALU.bypass,
                    )
                    # avg8 on the idle GPSIMD: keeps the DVE overlap
                    # window (next-rep scans vs this rep's gate products)
                    # under the gate-phase budget
                    nc.gpsimd.tensor_tensor(
                        avg8[:, dc, F16C:], cums[:, dc, F16C:],
                        inv8_t[:, F16C:], ALU.mult
                    )
                    nc.vector.tensor_tensor(
                        avgT[:, dc], cums[:, dc, 0:F16C], inv_t[:, 0:F16C],
                        ALU.mult
                    )
                return (avgT, avg8)

            pending = emit_attn()
            for rep in range(reps):
              avgT, avg8 = pending[0], pending[1]
              pre_w = pending[2:] if len(pending) > 2 else None
              out_rep = out_d if rep == 0 else scratch_d
              if "mm1" not in stages:
                continue
              # ---- mm1: hT/h8 = relu(W1^T @ avg + b1) -------------------
              # fp16 cols 0:F16C (W1 fp16 pre-scaled by S_A*S_W1 so the
              # psum scale is uniform), fp8 DR cols F16C:
              hT = big.tile([P, KC, F16C], f16, tag="hT")
              h8 = big.tile([P, KC, TOK], f8, tag="h8")
              inv_s1 = 1.0 / (S_A * S_W1)
              for q in range(KC // 2):
                if pre_w is not None and q < 2:
                    w16, w8 = pre_w[2 * q], pre_w[2 * q + 1]
                else:
                    w16 = wpool.tile([P, 2, KC, P], f16, tag="w16")
                    nc.sync.dma_start(w16[:], w116_d[q])
                    w8 = wpool.tile([P, 2, KC // 2, 2, P], f8, tag="w8")
                    nc.sync.dma_start(w8[:], w18_d[q])
                for m in range(2):
                    mc = 2 * q + m
                    pss = [
                        mm_ps.tile([P, NT], f32, tag="mps", name=f"mps{i}")
                        for i in range(NN)
                    ]
                    for kc in range(KC):
                        nc.tensor.matmul(
                            pss[0][:, 0:F16C],
                            w16[:, m, kc],
                            avgT[:, kc],
                            start=(kc == 0),
                            stop=(kc == KC - 1),
                        )
                    for j in range(KC // 2):
                        nc.tensor.matmul(
                            pss[0][:, F16C:NT],
                            w8[:, m, j],
                            avg8[:, 2 * j : 2 * j + 2, F16C:NT],
                            start=False,
                            stop=(j == KC // 2 - 1),
                            perf_mode=DR,
                            skip_group_check=True,
                        )
                        nc.tensor.matmul(
                            pss[1][:],
                            w8[:, m, j],
                            avg8[:, 2 * j : 2 * j + 2, NT : 2 * NT],
                            start=(j == 0),
                            stop=(j == KC // 2 - 1),
                            perf_mode=DR,
                            skip_group_check=True,
                        )
                    nc.scalar.activation(
                        hT[:, mc], pss[0][:, 0:F16C], AF.Relu,
                        scale=inv_s1, bias=b1_col(0, mc),
                    )
                    nc.scalar.activation(
                        h8[:, mc, F16C:NT], pss[0][:, F16C:NT], AF.Relu,
                        scale=inv_s1 * S_H, bias=b1_col(1, mc),
                    )
                    nc.scalar.activation(
                        h8[:, mc, NT : 2 * NT], pss[1][:], AF.Relu,
                        scale=inv_s1 * S_H, bias=b1_col(1, mc),
                    )

              if "mm2" not in stages:
                continue
              # ---- mm2: ffnT/ffn8 = W2^T @ h + b2 ----------------------
              ffnT = big.tile([P, KC, TOK], f16, tag="ffnT")
              ffn8 = big.tile([P, KC, TOK], f8, tag="ffn8")
              inv_s2 = 1.0 / (S_H * S_W2)
              for q in range(KC // 2):
                w16 = wpool.tile([P, 2, KC, P], f16, tag="w16")
                nc.sync.dma_start(w16[:], w216_d[q])
                w8 = wpool.tile([P, 2, KC // 2, 2, P], f8, tag="w8")
                nc.sync.dma_start(w8[:], w28_d[q])
                for m in range(2):
                    mc = 2 * q + m
                    pss = [
                        mm_ps.tile([P, NT], f32, tag="mps", name=f"mps{i}")
                        for i in range(NN)
                    ]
                    for kc in range(KC):
                        nc.tensor.matmul(
                            pss[0][:, 0:F16C],
                            w16[:, m, kc],
                            hT[:, kc],
                            start=(kc == 0),
                            stop=(kc == KC - 1),
                        )
                    for j in range(KC // 2):
                        nc.tensor.matmul(
                            pss[0][:, F16C:NT],
                            w8[:, m, j],
                            h8[:, 2 * j : 2 * j + 2, F16C:NT],
                            start=False,
                            stop=(j == KC // 2 - 1),
                            perf_mode=DR,
                            skip_group_check=True,
                        )
                        nc.tensor.matmul(
                            pss[1][:],
                            w8[:, m, j],
                            h8[:, 2 * j : 2 * j + 2, NT : 2 * NT],
                            start=(j == 0),
                            stop=(j == KC // 2 - 1),
                            perf_mode=DR,
                            skip_group_check=True,
                        )
                    for n in range(NN):
                        nsl = slice(n * NT, (n + 1) * NT)
                        nc.scalar.activation(
                            ffnT[:, mc, nsl], pss[n][:], AF.Identity,
                            scale=inv_s2, bias=b2_col(0, mc),
                        )
                    # ffn8 on DVE keeps Act off the critical path
                    nc.vector.tensor_scalar(
                        ffn8[:, mc], ffnT[:, mc], float(S_F), None, ALU.mult
                    )

              # gate-phase inputs: first dc's gate weights ahead of the big
              # activation loads so the gate start isn't DMA-gated
              iQT = big.tile([P, KC, TOK], f16, tag="iQT")
              iqh8 = big.tile([P, KC, TOK], f8, tag="iqh8")
              wg_pre = []
              for nm, dram in (("h", wgh_d), ("l", wgl_d), ("b", wgb_d)):
                w = wpool.tile([P, 2, KC // 2, 2, P], f8, tag=f"wgpre{nm}",
                               bufs=1, name=f"wgp{nm}")
                nc.sync.dma_start(w[:], dram[0])
                wg_pre.append(w)
              iqh_r = iqh_d.rearrange("(dc p) n -> p dc n", p=P)
              nc.sync.dma_start(iqh8[:], iqh_r[:])
              iq_r = iq_d.rearrange("(dc p) n -> p dc n", p=P)
              nc.sync.dma_start(iQT[:], iq_r[:])

              if "gate" not in stages:
                continue
              # ---- gate + final: per dc, gc pair (dc, dc+KC) -----------
              # one psum group per (gc, n): 8 hi + 8 lo iQ DR + 8 ffn DR,
              # all at psum scale 4096; Sigmoid evicts with scale 1/4096.
              for dc in range(KC):
                if dc == 0:
                    wgh, wgl, wgb = wg_pre
                else:
                    wgh = wpool.tile([P, 2, KC // 2, 2, P], f8, tag="wgh")
                    nc.sync.dma_start(wgh[:], wgh_d[dc])
                    wgl = wpool.tile([P, 2, KC // 2, 2, P], f8, tag="wgl")
                    nc.sync.dma_start(wgl[:], wgl_d[dc])
                    wgb = wpool.tile([P, 2, KC // 2, 2, P], f8, tag="wgb")
                    nc.sync.dma_start(wgb[:], wgb_d[dc])
                # software pipeline: mid-gate, queue the NEXT rep's attn
                # on the DVE so its scans overlap this rep's gate matmuls,
                # and issue its first mm1 weight DMAs so the SP queue has
                # them in flight before this rep's tail out-stores
                if dc == 2 and rep + 1 < reps:
                    pending = emit_attn()
                    pw = []
                    for qq in range(2):
                        w16p = wpool.tile([P, 2, KC, P], f16, tag="w16")
                        nc.sync.dma_start(w16p[:], w116_d[qq])
                        w8p = wpool.tile([P, 2, KC // 2, 2, P], f8,
                                         tag="w8")
                        nc.sync.dma_start(w8p[:], w18_d[qq])
                        pw += [w16p, w8p]
                    pending = pending + tuple(pw)
                g2 = gpool.tile([P, 2, TOK], f16, tag="g")
                outc = tokp.tile([P, TOK], f16, tag="tok")
                tmp = tokp.tile([P, TOK], f16, tag="tok")
                for gi in range(2):
                    gc = dc + gi * KC
                    pss = [
                        mm_ps.tile([P, NT], f32, tag="mps", name=f"mps{i}")
                        for i in range(NN)
                    ]
                    # per weight chunk j: one Ld serves the n0/n1 pair
                    # (post-compile dedup drops the second reload)
                    for ti, wq in enumerate((wgh, wgl)):
                        for j in range(KC // 2):
                            for n in range(NN):
                                nsl = slice(n * NT, (n + 1) * NT)
                                nc.tensor.matmul(
                                    pss[n][:],
                                    wq[:, gi, j],
                                    iqh8[:, 2 * j : 2 * j + 2, nsl],
                                    start=(ti == 0 and j == 0),
                                    stop=False,
                                    perf_mode=DR,
                                    skip_group_check=True,
                                )
                    for j in range(KC // 2):
                        for n in range(NN):
                            nsl = slice(n * NT, (n + 1) * NT)
                            nc.tensor.matmul(
                                pss[n][:],
                                wgb[:, gi, j],
                                ffn8[:, 2 * j : 2 * j + 2, nsl],
                                start=False,
                                stop=(j == KC // 2 - 1),
                                perf_mode=DR,
                                skip_group_check=True,
                            )
                    # last chunk's fgate sigmoid at 256-col granularity so
                    # the tail's DVE chain starts as soon as possible
                    sub = 2 if (dc == KC - 1 and gi == 1) else 1
                    for n in range(NN):
                        for k in range(sub):
                            w = NT // sub
                            nsl = slice(n * NT + k * w, n * NT + (k + 1) * w)
                            nc.scalar.activation(
                                g2[:, gi, nsl], pss[n][:, k * w : (k + 1) * w],
                                AF.Sigmoid,
                                scale=1.0 / SCALE_G, bias=bg_col(gc),
                            )
                        # igate*iQ overlaps the fgate matmuls on DVE
                        if gi == 0:
                            nsl = slice(n * NT, (n + 1) * NT)
                            nc.vector.tensor_tensor(
                                outc[:, nsl], g2[:, 0, nsl],
                                iQT[:, dc, nsl], ALU.mult
                            )
                # last chunk runs at half-NT granularity to shorten the
                # sigmoid -> DVE -> DMA tail; earlier chunks batch into one
                # store
                steps = 2 * NN if dc == KC - 1 else NN
                W = TOK // steps
                for n in range(steps):
                    nsl = slice(n * W, (n + 1) * W)
                    nc.vector.tensor_tensor(
                        tmp[:, nsl], g2[:, 1, nsl], ffnT[:, dc, nsl],
                        ALU.mult
                    )
                    nc.vector.tensor_tensor(
                        outc[:, nsl], outc[:, nsl], tmp[:, nsl], ALU.add
                    )
                    if dc == KC - 1:
                        nc.sync.dma_start(
                            out_rep[dc * P : (dc + 1) * P, nsl],
                            outc[:, nsl],
                        )
                if dc < KC - 1:
                    nc.sync.dma_start(
                        out_rep[dc * P : (dc + 1) * P, :], outc[:]
                    )

    nc.compile()
    _dedup_ldweights(nc)
    return nc


def _get_nc(reps=1, stages=("attn", "mm1", "mm2", "gate")):
    key = ("nc", reps, tuple(stages))
    if key not in _CACHE:
        _CACHE[key] = _build_nc(reps, stages)
    return _CACHE[key]


def _pack_pairs(W, s):
    """[D_in, P*mc] weight chunk -> fp8 [mc, P(k), pairs, 2, P(m)] * s."""
    din, dout = W.shape
    kc, mc = din // P, dout // P
    r = (W * s).reshape(kc // 2, 2, P, mc, P)  # (pair, i, k, mc, m)
    r = np.ascontiguousarray(r.transpose(3, 2, 0, 1, 4))  # mc, k, pair, i, m
    return r.astype(E4)


def _pack_pairs_pre(Wq):
    """Pre-quantized [D_in, D_out] -> fp8 [mc, P(k), pairs, 2, P(m)]."""
    din, dout = Wq.shape
    kc, mc = din // P, dout // P
    r = Wq.reshape(kc // 2, 2, P, mc, P)
    r = np.ascontiguousarray(r.transpose(3, 2, 0, 1, 4))
    return r.astype(E4)


def _rearr16(W, s=1.0):
    """[D_in, P*mc] -> fp16 [mc, P(k), kc, P(m)] (lhsT chunk layout)."""
    din, dout = W.shape
    kc, mc = din // P, dout // P
    r = (W * s).reshape(kc, P, mc, P).transpose(2, 1, 0, 3)
    return np.ascontiguousarray(r).astype(np.float16)


def _mc_pair(a):
    """[mc, P, ...] -> [mc//2, P, 2, ...] (adjacent output-chunk pairs)."""
    m = a.shape[0]
    r = a.reshape(m // 2, 2, *a.shape[1:])
    return np.ascontiguousarray(np.swapaxes(r, 1, 2))


def _gc_pair(a):
    """[gc(16), P, ...] -> [dc(8), P, 2, ...] pairing (dc, dc+KC)."""
    r = a.reshape(2, KC, *a.shape[1:])            # [gi, dc, P, ...]
    r = np.moveaxis(r, 0, 2)                      # [dc, P, gi, ...]
    return np.ascontiguousarray(r)


def _host_inputs(iQ, iV, W1, b1, W2, b2, Wg, bg):
    iQ = np.asarray(iQ, np.float32)
    iV = np.asarray(iV, np.float32)
    W1 = np.asarray(W1, np.float32)
    W2 = np.asarray(W2, np.float32)
    Wg = np.asarray(Wg, np.float32)

    w116 = _mc_pair(_rearr16(W1, S_A * S_W1))
    w18 = _mc_pair(_pack_pairs(W1, S_W1))
    w216 = _mc_pair(_rearr16(W2, S_H * S_W2))
    w28 = _mc_pair(_pack_pairs(W2, S_W2))
    wgt_s = Wg[:D] * S_WGT
    wgth_v = wgt_s.astype(E4)
    wgh = _gc_pair(_pack_pairs_pre(wgth_v.astype(np.float32)))
    wgl = _gc_pair(_pack_pairs_pre(
        (wgt_s - wgth_v.astype(np.float32)).astype(E4).astype(np.float32)
    ))
    wgb = _gc_pair(_pack_pairs(Wg[D:], S_WG))

    def bias2(b, s):
        bc = np.asarray(b, np.float32).reshape(KC, P).T  # [P, KC]
        return np.stack([bc, bc * s], axis=1)  # [P, 2, KC]

    b1c = bias2(b1, S_H)
    b2c = bias2(b2, S_F)
    bgc = np.asarray(bg, np.float32).reshape(GC, P).T

    in_maps = []
    for c in range(NCORES):
        b, h = divmod(c, 2)
        sl = slice(h * TOK, (h + 1) * TOK)
        inv = np.float32(1.0) / np.arange(
            h * TOK + 1, h * TOK + TOK + 1, dtype=np.float32
        )
        carry0 = (
            iV[b, :TOK].astype(np.float64).sum(axis=0).astype(np.float32)
            if h
            else np.zeros(D, np.float32)
        )
        c16 = np.empty((P, C16_W), np.float16)
        c16[:, 0:TOK] = inv.astype(np.float16)[None, :]
        c16[:, TOK:] = (inv * S_A).astype(np.float16)[None, :]
        c32 = np.empty((P, C32_W), np.float32)
        c32[:, 0:KC] = carry0.reshape(KC, P).T
        c32[:, KC : 3 * KC] = b1c.reshape(P, 2 * KC)
        c32[:, 3 * KC : 5 * KC] = b2c.reshape(P, 2 * KC)
        c32[:, 5 * KC :] = bgc
        # V feature-major f16: ivt[p, kc, t] = iV[b, sl][t, kc*P+p]
        ivt = np.ascontiguousarray(
            iV[b, sl].T.reshape(KC, P, TOK).transpose(1, 0, 2)
        ).astype(np.float16)
        iqs = iQ[b, sl].T * S_IQ  # [D, TOK]
        in_maps.append(
            {
                "iq": np.ascontiguousarray(iQ[b, sl].T).astype(np.float16),
                "iqh": np.ascontiguousarray(iqs.astype(E4)),
                "ivt": ivt,
                "w116": w116,
                "w18": w18,
                "w216": w216,
                "w28": w28,
                "wgh": wgh,
                "wgl": wgl,
                "wgb": wgb,
                "c16": c16,
                "c32": c32,
            }
        )
    return in_maps


def _gather(results):
    out = np.empty((B, S, D), np.float32)
    for c in range(NCORES):
        b, h = divmod(c, 2)
        out[b, h * TOK : (h + 1) * TOK, :] = (
            results[c]["outT"].astype(np.float32).T
        )
    return out


def kernel(iQ, iV, W1, b1, W2, b2, Wg, bg):
    from concourse.bass_utils import run_bass_kernel_spmd

    nc = _get_nc()
    in_maps = _host_inputs(iQ, iV, W1, b1, W2, b2, Wg, bg)
    res = run_bass_kernel_spmd(nc, in_maps, core_ids=list(range(NCORES)))
    return _gather(res.results)
